# revision 19
# baseline (speedup 1.0000x reference)
"""MDTA Bass kernel for 8 TRN2 NeuronCores, two SPMD launches.

Math (row-major reshape of the reference):
  q.reshape(B,HEADS,HW,D) maps tensor[b,hd,s,d] = conv[b,16hd+ci,y,16xs+d],
  s = ci*1024 + y*8 + xs, so the attention feature axis d is x%16 and
  G[hd,d,j] = sum_{ci,y,xs} k2[16hd+ci,y,16xs+d] * q2[16hd+ci,y,16xs+j]
  out_conv[b,16hd+j, ci*8+y//16, (y%16)*8+xs] = sum_d v2[16hd+ci,y,16xs+d]*P[d,j]

Key structural choices vs a naive port:
  * conv1x1+conv3x3 compose into ONE 3x3 conv (both linear): w2' = w2 @ w1
    precomputed on host; the device runs a single 9-tap matmul chain per
    q/k/v. The wo 1x1 conv folds into the attention-apply matmul the same
    way (M = PSTACK @ wo^T on host), and the residual add happens on host.
  * Score path (q,k convs, transposes, pair matmuls) stays fp32r: the
    logits have magnitude ~1e2 and softmax amplifies absolute G errors, so
    bf16 there costs ~1e-2 rel err. The v path is insensitive (smooth
    averaging) and runs bf16 from the PSUM copy onward, as does launch 2.
  * PSUM->SBUF copies round-robin across DVE/Act/Pool so no single engine
    bottlenecks behind the PE.

Launch 1 (spatial shards: b x quarter-of-H, 1-row halo): LayerNorm, three
9-tap convs, two PE-transpose levels for q/k, per-head 128x128 pair
matrices. Outputs v2 (bf16) + pairs (f32). Host: strip-diagonal gram,
softmax, M = PSTACK @ wo^T, VROW shuffle of v2 (pure byte movement).
Launch 2: 32 matmuls [128,128] (M^T @ VROW = wo(attention) rows, bf16).
Host: upcast + residual.
"""

import os
from contextlib import ExitStack

import numpy as np

import concourse.bacc as bacc
import concourse.bass as bass
import concourse.mybir as mybir
import concourse.tile as tile
from concourse import bass_utils

F32 = mybir.dt.float32
F32R = mybir.dt.float32r
BF16 = mybir.dt.bfloat16
AX = mybir.AxisListType
ALU = mybir.AluOpType
ACT = mybir.ActivationFunctionType

NPBF16 = mybir.dt.np(BF16)

B, C, H, W = 2, 128, 128, 128
HEADS, D = 8, 16
EPS = 1e-5
RPC = H // 4          # output rows per core
RH = RPC + 2          # with 1-row halo each side
NPIX = RPC * W        # 4096
NHAL = RH * W         # 4352
WP = W + 2            # padded width

_CACHE = {}

# score-path precision: "f32r" (safe) or "bf16" (faster; one extra rounding
# of the q/k conv outputs feeding the pair matmuls)
SCORE = os.environ.get("KERNEL_SCORE", "f32r")


def _build_l1(affine):
    bf = SCORE == "bf16"
    SD = BF16 if bf else F32

    def sc(ap):
        return ap if bf else ap.bitcast(F32R)

    def so(ap):
        return ap if bf else ap.bitcast(F32R)

    nc = bacc.Bacc("TRN2", target_bir_lowering=False, debug=False, num_devices=8)
    x_d = nc.dram_tensor("x_sl", [128, 2 * RH + NHAL], F32,
                         kind="ExternalInput").ap()
    w_d = {t: nc.dram_tensor(f"w{t}", [128, 9 * 128], F32R,
                         kind="ExternalInput").ap()
           for t in "qkv"}
    idn_d = nc.dram_tensor("ident", [128, 128], BF16 if bf else F32R,
                         kind="ExternalInput").ap()
    if affine:
        gm_d = nc.dram_tensor("gamma_b", [128, W], F32, kind="ExternalInput").ap()
        bt_d = nc.dram_tensor("beta_b", [128, W], F32, kind="ExternalInput").ap()
    v2_d = nc.dram_tensor("v2o", [128, NPIX], BF16, kind="ExternalOutput").ap()
    pr_d = nc.dram_tensor("pairs", [128, 8 * 128], F32, kind="ExternalOutput").ap()

    with tile.TileContext(nc) as tc, ExitStack() as ctx:
        consts = ctx.enter_context(tc.tile_pool(name="consts", bufs=1))
        big = ctx.enter_context(tc.tile_pool(name="big", bufs=1))
        sbw = ctx.enter_context(tc.tile_pool(name="sbw", bufs=3))
        ps_cv = ctx.enter_context(tc.tile_pool(name="ps_cv", bufs=2, space="PSUM"))
        ps_t1 = ctx.enter_context(tc.tile_pool(name="ps_t1", bufs=2, space="PSUM"))
        ps_t2 = ctx.enter_context(tc.tile_pool(name="ps_t2", bufs=2, space="PSUM"))
        ps_pr = ctx.enter_context(tc.tile_pool(name="ps_pr", bufs=2, space="PSUM"))

        # PSUM drains alternate DVE/Act (GPSIMD cannot access PSUM)
        cpe = [nc.vector.tensor_copy,
               lambda o, i: nc.scalar.copy(o, i)]
        cpi = [0]

        def cp(out, in_):
            cpe[cpi[0] % 2](out, in_)
            cpi[0] += 1

        # ---- hoist the Act func-table load off the critical path ----
        dmy = sbw.tile([128, 1], F32, name="dmy", tag="dmy", bufs=1)
        nc.vector.memset(dmy[:], 0.0)
        nc.scalar.copy(dmy[:], dmy[:])

        # ---- inputs: ln + first x chunk + q-weights lead; f32r rounding
        # happens inside the PE, so staged fp32 weights are bitcast directly
        # x_t holds [rstd | nmr | x-rows]: one leading DMA covers the LN
        # scalars and the first conv rows
        LNC = 2 * RH
        x_t = big.tile([128, LNC + NHAL], F32, name="x_t", tag="x_t")
        ln = x_t
        chunks = ((0, 6), (6, 15), (15, 24), (24, 34))
        w = {}
        for t in "qkv":
            w[t] = consts.tile([128, 9 * 128], F32R, name=f"w{t}r", tag=f"w{t}r")
        nc.sync.dma_start(x_t[:, 0:LNC + chunks[0][1] * W],
                          x_d[:, 0:LNC + chunks[0][1] * W])
        for i in range(3):
            nc.sync.dma_start(w["q"][:, i * 384:(i + 1) * 384],
                              w_d["q"][:, i * 384:(i + 1) * 384])
        for a, b_ in chunks[1:]:
            nc.sync.dma_start(x_t[:, LNC + a * W:LNC + b_ * W],
                              x_d[:, LNC + a * W:LNC + b_ * W])
        for t in "kv":
            nc.sync.dma_start(w[t][:], w_d[t][:])
        idn = consts.tile([128, 128], BF16 if bf else F32R, name="idn",
                          tag="idn")
        nc.sync.dma_start(idn[:], idn_d[:])
        if affine:
            gm = consts.tile([128, W], F32, name="gm", tag="gm")
            nc.sync.dma_start(gm[:], gm_d[:])
            bt = consts.tile([128, W], F32, name="bt", tag="bt")
            nc.sync.dma_start(bt[:], bt_d[:])

        # ---- normalize (host-computed rstd/nmr), pipelined per x-chunk ----
        xn = big.tile([128, RH * WP], F32, name="xn", tag="xn")
        xnv = xn.rearrange("p (r w) -> p r w", w=WP)
        zpad = sbw.tile([128, RH], F32, name="zpad", tag="zpad", bufs=1)
        nc.vector.memset(zpad[:], 0.0)
        zp3 = zpad.rearrange("p (r o) -> p r o", o=1)
        nc.vector.tensor_copy(xnv[:, :, 0:1].bitcast(F32R), zp3)
        nc.vector.tensor_copy(xnv[:, :, WP - 1:WP].bitcast(F32R), zp3)
        for a, b_ in chunks:
            for r in range(a, b_):
                dst = xnv[:, r, 1:1 + W]
                src = x_t[:, LNC + r * W:LNC + (r + 1) * W]
                if r % 2 == 0:
                    nc.scalar.activation(dst.bitcast(F32R), src, ACT.Identity,
                                         bias=ln[:, RH + r:RH + r + 1],
                                         scale=ln[:, r:r + 1])
                else:
                    nc.gpsimd.tensor_scalar(dst.bitcast(F32R), src,
                                            ln[:, r:r + 1],
                                            ln[:, RH + r:RH + r + 1],
                                            op0=ALU.mult, op1=ALU.add)
                if affine:
                    nc.vector.tensor_tensor(dst.bitcast(F32R), dst, gm[:],
                                            op=ALU.mult)
                    nc.gpsimd.tensor_tensor(dst.bitcast(F32R), dst, bt[:],
                                            op=ALU.add)

        # ---- 9-tap convs (composed weights) ----
        c2 = {}

        def conv_group(t, g, sink):
            ps = ps_cv.tile([128, 512], F32, name=f"cv{t}{g}", tag="cv")
            for off in range(9):
                dy, dx = off // 3, off % 3
                rhs = xnv[:, 4 * g + dy:4 * g + dy + 4, dx:dx + 128]
                nc.tensor.matmul(ps[:], w[t][:, off * 128:(off + 1) * 128],
                                 rhs.bitcast(F32R), start=(off == 0), stop=(off == 8))
            sink(g, ps)

        def qk_sink(t):
            dst = big.tile([128, NPIX], SD, name=f"c2{t}", tag=f"c2{t}")
            c2[t] = dst

            def sink(g, ps):
                cp(so(dst[:, g * 512:(g + 1) * 512]), ps[:])
            return sink

        def v_sink(g, ps):
            vst = sbw.tile([128, 512], BF16, name=f"vst{g}", tag="vst", bufs=3)
            cp(vst[:], ps[:])
            nc.sync.dma_start(v2_d[:, g * 512:(g + 1) * 512], vst[:])

        # ---- level-1 transpose: tb[x, (hd,o,y8,ci)] = c2[16hd+ci, (8o+y8)*128+x]
        tbig = {}

        def t1_group(t, gq):
            tb5 = tbig[t].rearrange("p (h o y c) -> p h o y c", h=8, o=4, y=8)
            psT = ps_t1.tile([128, 512], F32, name=f"t1{t}{gq}", tag="t1")
            for i in range(4):
                y = 4 * gq + i
                nc.tensor.transpose(psT[:, i * 128:(i + 1) * 128].bitcast(F32R),
                                    sc(c2[t][:, y * 128:(y + 1) * 128]),
                                    sc(idn))
            o, ys = gq // 2, 4 * (gq % 2)
            src = psT.rearrange("p (r h c) -> p h r c", r=4, h=8)
            cp(so(tb5[:, :, o, ys:ys + 4, :]), src)

        # ---- level-2 transpose + per-head pair matmul, interleaved with v ----
        pair_sb = big.tile([128, 8 * 128], F32, name="pair_sb", tag="pair_sb")
        t4 = {}
        pps = {}

        def t2_head(hd):
            for t in "qk":
                tb5 = tbig[t].rearrange("p (h f) -> p h f", h=8)
                sb = sbw.tile([128, 512], SD, name=f"t4{t}{hd}", tag=f"t4{t}", bufs=2)
                if bf:
                    # 16-bit transpose runs on the DMA engines' XBAR: no PE,
                    # no PSUM, no drain copy
                    for o in range(4):
                        nc.sync.dma_start(sb[:, o * 128:(o + 1) * 128],
                                          tb5[:, hd, o * 128:(o + 1) * 128],
                                          transpose=True)
                else:
                    psT2 = ps_t2.tile([128, 512], F32, name=f"t2{t}{hd}", tag="t2")
                    for o in range(4):
                        nc.tensor.transpose(psT2[:, o * 128:(o + 1) * 128].bitcast(F32R),
                                            sc(tb5[:, hd, o * 128:(o + 1) * 128]),
                                            sc(idn))
                    cp(so(sb[:]), psT2[:])
                t4[t, hd] = sb

        def pair_head(hd):
            pp = ps_pr.tile([128, 128], F32, name=f"pps{hd}", tag="pps")
            for o in range(4):
                nc.tensor.matmul(pp[:], sc(t4["k", hd][:, o * 128:(o + 1) * 128]),
                                 sc(t4["q", hd][:, o * 128:(o + 1) * 128]),
                                 start=(o == 0), stop=(o == 3))
            cp(pair_sb[:, hd * 128:(hd + 1) * 128], pp[:])
            nc.sync.dma_start(pr_d[:, hd * 128:(hd + 1) * 128],
                              pair_sb[:, hd * 128:(hd + 1) * 128])

        qs = qk_sink("q")
        tbig["q"] = big.tile([128, NPIX], SD, name="tbq", tag="tbq")
        ks = qk_sink("k")
        tbig["k"] = big.tile([128, NPIX], SD, name="tbk", tag="tbk")
        for g in range(8):
            conv_group("q", g, qs)
        for g in range(8):
            t1_group("q", g)
        for g in range(8):
            conv_group("k", g, ks)
        for g in range(8):
            t1_group("k", g)
        # v groups interleaved with per-head transpose+pair blocks: the v conv
        # keeps the PE busy while the pair operands drain from PSUM, and the
        # last pair's copy/DMA tail hides under the last v group.
        t2_head(0)
        t2_head(1)
        for i in range(8):
            pair_head(i)
            conv_group("v", i, v_sink)
            if i < 6:
                t2_head(i + 2)

    nc.compile()
    return nc


def _build_l2():
    nc = bacc.Bacc("TRN2", target_bir_lowering=False, debug=False, num_devices=8)
    m_d = nc.dram_tensor("mT", [128, 128], BF16, kind="ExternalInput").ap()
    vr_d = nc.dram_tensor("vrows", [128, NPIX], BF16, kind="ExternalInput").ap()
    y_d = nc.dram_tensor("y_att", [128, NPIX], BF16, kind="ExternalOutput").ap()

    with tile.TileContext(nc) as tc, ExitStack() as ctx:
        consts = ctx.enter_context(tc.tile_pool(name="consts", bufs=1))
        big = ctx.enter_context(tc.tile_pool(name="big", bufs=1))
        sbw = ctx.enter_context(tc.tile_pool(name="sbw", bufs=3))
        ps_a = ctx.enter_context(tc.tile_pool(name="ps_a", bufs=2, space="PSUM"))

        cpe = [nc.vector.tensor_copy,
               lambda o, i: nc.scalar.copy(o, i),
               nc.gpsimd.tensor_copy]
        cpi = [0]

        vr = big.tile([128, NPIX], BF16, name="vr", tag="vr")
        mT = consts.tile([128, 128], BF16, name="mT", tag="mT")
        # ascending chunk sizes: a small first chunk starts the matmuls early,
        # bigger later chunks amortize the per-DMA latency
        nc.sync.dma_start(vr[:, 0:256], vr_d[:, 0:256])
        nc.sync.dma_start(mT[:], m_d[:])
        for a, b_ in ((256, 1280), (1280, 2560), (2560, 4096)):
            nc.sync.dma_start(vr[:, a:b_], vr_d[:, a:b_])

        yst = big.tile([128, NPIX], BF16, name="yst", tag="yst")
        for g in range(8):
            ps = ps_a.tile([128, 512], F32, name=f"ar{g}", tag="ar")
            for i in range(4):
                y = 4 * g + i
                nc.tensor.matmul(ps[:, i * 128:(i + 1) * 128], mT[:],
                                 vr[:, y * 128:(y + 1) * 128],
                                 start=True, stop=True)
            seg = slice(g * 512, (g + 1) * 512)
            cpe[cpi[0] % 2](yst[:, seg], ps[:])
            cpi[0] += 1
            if g % 2 == 1:
                nc.sync.dma_start(y_d[:, (g - 1) * 512:(g + 1) * 512],
                                  yst[:, (g - 1) * 512:(g + 1) * 512])

    nc.compile()
    return nc


def _get(name, affine=False):
    key = (name, affine, SCORE)
    if key not in _CACHE:
        _CACHE[key] = _build_l1(affine) if name == "l1" else _build_l2()
    return _CACHE[key]


def _host_middle(pairs_list, scale, wo2d):
    """pairs -> gram -> softmax P -> M = PSTACK @ wo^T per batch (bf16)."""
    f = np.float32
    G = np.zeros((B, HEADS, D, D), f)
    for c in range(8):
        pr = pairs_list[c].reshape(128, 8, 128)
        for hd in range(HEADS):
            blk = pr[:, hd, :].reshape(8, 16, 8, 16)      # [xs, d, xs', j]
            G[c // 4, hd] += np.einsum("adaj->dj", blk)
    G /= float(np.asarray(scale, f)[0])
    Gm = G - G.max(-1, keepdims=True)
    E = np.exp(Gm)
    P = (E / E.sum(-1, keepdims=True)).astype(f)          # [B, HEADS, 16, 16]

    ms = []
    for b in range(B):
        pstack = np.zeros((128, 128), f)
        for hd in range(HEADS):
            pstack[16 * hd:16 * hd + 16, 16 * hd:16 * hd + 16] = P[b, hd]
        ms.append(np.ascontiguousarray(pstack @ wo2d.T).astype(NPBF16))
    return ms


def _vrows(v2o_list):
    """v2 (bf16, conv layout) -> per-core VROW matrices (pure indexing)."""
    v_conv = np.empty((B, C, H, W), NPBF16)
    for c in range(8):
        b, r0 = c // 4, 32 * (c % 4)
        v_conv[b, :, r0:r0 + RPC, :] = v2o_list[c].reshape(C, RPC, W)
    # vc[b, hd, ci, y, xs, d]
    vc = v_conv.reshape(B, HEADS, 16, H, 8, 16)
    vrows = []
    for c in range(8):
        b, r0 = c // 4, 32 * (c % 4)
        rows = np.empty((32, 128, 128), NPBF16)
        for i in range(32):
            yp = r0 + i
            ci, yb = yp // 8, yp % 8
            blk = vc[b, :, ci, 16 * yb:16 * yb + 16, :, :]   # [hd, yy, xs, d]
            rows[i] = blk.transpose(0, 3, 1, 2).reshape(128, 128)
        vrows.append(np.ascontiguousarray(rows.transpose(1, 0, 2))
                     .reshape(128, NPIX))
    return vrows


def _maps_l1(x, gamma, beta, wq1, wq2, wk1, wk2, wv1, wv2, affine):
    f = np.float32
    xp = np.pad(np.asarray(x, f), ((0, 0), (0, 0), (1, 1), (0, 0)))
    # host-side LayerNorm stats over the padded rows (pad rows: mu=0, xn=0)
    mu = xp.mean(-1)
    var = xp.var(-1)
    rstd = (1.0 / np.sqrt(var + EPS)).astype(f)
    nmr = (-mu * rstd).astype(f)
    common = {"ident": np.eye(128, dtype=f if SCORE == "f32r" else NPBF16)}
    if affine:
        common["gamma_b"] = np.broadcast_to(np.asarray(gamma, f), (128, W)).copy()
        common["beta_b"] = np.broadcast_to(np.asarray(beta, f), (128, W)).copy()
    for t, w1_, w2_ in (("q", wq1, wq2), ("k", wk1, wk2), ("v", wv1, wv2)):
        w1n = np.asarray(w1_, f)[:, :, 0, 0]               # [cm, ci]
        w2n = np.asarray(w2_, f)                           # [co, cm, 3, 3]
        w2p = np.einsum("omyx,mi->oiyx", w2n, w1n)         # composed 3x3
        common[f"w{t}"] = np.ascontiguousarray(
            w2p.transpose(1, 2, 3, 0).reshape(128, 9 * 128))
    maps = []
    for c in range(8):
        b, r0 = c // 4, 32 * (c % 4)
        m = dict(common)
        m["x_sl"] = np.ascontiguousarray(np.concatenate(
            [rstd[b, :, r0:r0 + RH], nmr[b, :, r0:r0 + RH],
             xp[b, :, r0:r0 + RH, :].reshape(128, NHAL)], axis=1))
        maps.append(m)
    return maps


def _run(nc, maps, key):
    trace = bool(int(os.environ.get("KERNEL_TRACE", "0")))
    if _CACHE.get("sim"):
        from concourse.bass_interp import MultiCoreSim
        sim = MultiCoreSim(nc, num_cores=8, require_finite=True, require_nnan=True)
        cores = list(sim.cores.values())
        for c, m in enumerate(maps):
            for k, v in m.items():
                cores[c].tensor(k)[:] = v
        sim.simulate(check_with_hw=False)
        return [{k: np.array(cores[c].tensor(k)) for k in key} for c in range(8)]
    res = bass_utils.run_bass_kernel_spmd(nc, maps, core_ids=list(range(8)),
                                          trace=trace)
    _CACHE.setdefault("results", []).append(res)
    return res.results


def kernel(x, gamma, beta, scale, wq1, wq2, wk1, wk2, wv1, wv2, wo):
    f = np.float32
    affine = not (np.all(np.asarray(gamma, f) == 1.0)
                  and np.all(np.asarray(beta, f) == 0.0))
    r1 = _run(_get("l1", affine),
              _maps_l1(x, gamma, beta, wq1, wq2, wk1, wk2, wv1, wv2, affine),
              ("v2o", "pairs"))
    wo2d = np.asarray(wo, f)[:, :, 0, 0]
    ms = _host_middle([r["pairs"] for r in r1], scale, wo2d)
    vrows = _vrows([r["v2o"] for r in r1])
    maps2 = [{"mT": ms[c // 4], "vrows": vrows[c]} for c in range(8)]
    r2 = _run(_get("l2"), maps2, ("y_att",))
    y = np.empty((B, C, H, W), f)
    for c in range(8):
        b, r0 = c // 4, 32 * (c % 4)
        y[b, :, r0:r0 + RPC, :] = r2[c]["y_att"].astype(f).reshape(C, RPC, W)
    return y + np.asarray(x, f)


def kernel_sim(**inputs):
    _CACHE["sim"] = True
    try:
        return kernel(**inputs)
    finally:
        _CACHE["sim"] = False
